# revision 13
# baseline (speedup 1.0000x reference)
"""RWKV time-mixing kernel for 8 Trainium2 NeuronCores (Bass/Tile).

Problem: B=4, T=4096, C=1024.
  k = x @ Wk.T; v = x @ Wv.T; r = sigmoid(x @ Wr.T)
  WKV recurrence over T per (b, channel), out = (r * wkv) @ Wo.T
  plus final recurrence states (aa, bb, pp).

Sharding: 8 cores = (batch b in 0..3) x (channel half h in 0..1).
Each core computes projections for its 512 output channels (contracting
over the full C=1024 input), runs the recurrence with the DVE
tensor_tensor_scan (state = lam*state + e^k*v per channel along T), and
emits a PARTIAL output matmul over its 512 channels. The host sums the
two partials per batch (exact fp32 add) and re-transposes.

Math: the reference's max-stabilized scan is replaced by the equivalent
unstabilized recurrence (safe in fp32 for these magnitudes):
  A_t = lam*A_{t-1} + e^{k_t} v_t,  B_t = lam*B_{t-1} + e^{k_t}
  wkv_t = (A_{t-1} + e^u e^{k_t} v_t) / (B_{t-1} + e^u e^{k_t})
The scan output is written shifted one column so A_{t-1}/B_{t-1} come
for free; num is a fused scalar_tensor_tensor op. den/wkv/rwkv run on
GPSIMD to keep the Vector engine under the PE roofline.
Final states: pp_t = max(pp_{t-1} - w, k_t) is itself a scan (add/max);
aa = A_T * e^{-pp_T}, bb = B_T * e^{-pp_T}.

Engine budget per core (analytic): PE ~230us (4 matmuls, bf16),
DVE ~190us (3 half-rate scans + pv/num/recip), GPSIMD ~125us,
ACT ~95us, DMA ~20MB. PE-bound.
"""

import os
import sys

if "/opt/trn_rl_repo" not in sys.path:
    sys.path.insert(0, "/opt/trn_rl_repo")

import numpy as np
import ml_dtypes

BF16 = ml_dtypes.bfloat16
F32 = np.float32

B, T, C = 4, 4096, 1024
NCORES = 8
CSH = C // 2          # channels per core (512)
P = 128               # partitions
NCT = CSH // P        # 4 channel tiles per core
NKT = C // P          # 8 contraction k-tiles
TB = 512              # T block (one PSUM bank)
NTB = T // TB         # 8
NTH = 1               # out-matmul phases (1 = single phase at the end)
NTBH = NTB // NTH     # blocks per phase
NDT = C // P          # 8 output d-tiles

_NC = None            # cached finalized Bass program


def _build_nc(loop_iters=None):
    import concourse.bacc as bacc
    import concourse.mybir as mybir
    import concourse.tile as tile
    from contextlib import ExitStack

    f32 = mybir.dt.float32
    bf16 = mybir.dt.bfloat16
    AF = mybir.ActivationFunctionType
    OP = mybir.AluOpType

    nc = bacc.Bacc("TRN2")

    xT_d = nc.dram_tensor("xT", [C, T], bf16, kind="ExternalInput")
    wkT_d = nc.dram_tensor("wkT", [C, CSH], bf16, kind="ExternalInput")
    wvT_d = nc.dram_tensor("wvT", [C, CSH], bf16, kind="ExternalInput")
    wrT_d = nc.dram_tensor("wrT", [C, CSH], bf16, kind="ExternalInput")
    woT_d = nc.dram_tensor("woT", [CSH, C], bf16, kind="ExternalInput")
    lam_d = nc.dram_tensor("lam", [CSH], f32, kind="ExternalInput")
    negw_d = nc.dram_tensor("negw", [CSH], f32, kind="ExternalInput")
    eu_d = nc.dram_tensor("eu", [CSH], f32, kind="ExternalInput")
    a0_d = nc.dram_tensor("a0", [CSH], f32, kind="ExternalInput")
    b0_d = nc.dram_tensor("b0", [CSH], f32, kind="ExternalInput")
    p0_d = nc.dram_tensor("p0", [CSH], f32, kind="ExternalInput")

    outT_d = nc.dram_tensor("outT", [C, T], bf16, kind="ExternalOutput")
    aa_d = nc.dram_tensor("aaS", [CSH], f32, kind="ExternalOutput")
    bb_d = nc.dram_tensor("bbS", [CSH], f32, kind="ExternalOutput")
    pp_d = nc.dram_tensor("ppS", [CSH], f32, kind="ExternalOutput")

    with ExitStack() as ctx:
        tc = ctx.enter_context(tile.TileContext(nc))
        cpool = ctx.enter_context(tc.tile_pool(name="const", bufs=1))
        xpool = ctx.enter_context(tc.tile_pool(name="x", bufs=1))
        wpool = ctx.enter_context(tc.tile_pool(name="w", bufs=2))
        wopool = ctx.enter_context(tc.tile_pool(name="wo", bufs=1))
        rwpool = ctx.enter_context(tc.tile_pool(name="rwkv", bufs=1))
        work = ctx.enter_context(tc.tile_pool(name="work", bufs=3))
        scanp = ctx.enter_context(tc.tile_pool(name="scan", bufs=2))
        stp = ctx.enter_context(tc.tile_pool(name="st", bufs=1))
        pproj = ctx.enter_context(tc.tile_pool(name="pproj", bufs=2, space="PSUM"))
        pout = ctx.enter_context(tc.tile_pool(name="pout", bufs=2, space="PSUM"))

        loop_cm = tc.For_i(0, loop_iters, 1) if loop_iters else None
        if loop_cm is not None:
            ctx.enter_context(loop_cm)

        # per-channel constants as [128, NCT] (col = channel tile)
        def load_const(d, tag):
            t = cpool.tile([P, NCT], f32, tag=tag, name=tag)
            nc.sync.dma_start(t[:], d[:].rearrange("(j p) -> p j", p=P))
            return t

        abl_no_out = bool(os.environ.get("ABL_NO_OUT"))
        abl_pe_only = bool(os.environ.get("ABL_PE_ONLY"))
        abl_scan_copy = bool(os.environ.get("ABL_SCAN_COPY"))

        lam_s = load_const(lam_d, "lam")
        eu_s = load_const(eu_d, "eu")
        a0_s = load_const(a0_d, "a0")
        b0_s = load_const(b0_d, "b0")
        m0_s = load_const(p0_d, "m0")   # host passes m0 = e^{pp0} here

        # resident xT (8 k-tiles of [128, T])
        xts = []
        for k in range(NKT):
            xt = xpool.tile([P, T], bf16, tag=f"xt{k}", name=f"xt{k}")
            nc.sync.dma_start(xt[:], xT_d[k * P:(k + 1) * P, :])
            xts.append(xt)

        # resident output weights per channel tile: [128c, 1024d]
        wos = []
        for ct in range(NCT):
            wo = wopool.tile([P, C], bf16, tag=f"wo{ct}", name=f"wo{ct}")
            nc.sync.dma_start(wo[:], woT_d[ct * P:(ct + 1) * P, :])
            wos.append(wo)

        # carry tiles (recurrence state at half boundaries / end)
        carryA = [stp.tile([P, 1], f32, tag=f"cA{ct}", name=f"cA{ct}")
                  for ct in range(NCT)]
        carryB = [stp.tile([P, 1], f32, tag=f"cB{ct}", name=f"cB{ct}")
                  for ct in range(NCT)]
        carryP = [stp.tile([P, 1], f32, tag=f"cP{ct}", name=f"cP{ct}")
                  for ct in range(NCT)]

        # final state collection tiles
        aa_sb = stp.tile([P, NCT], f32, tag="aa", name="aa_sb")
        bb_sb = stp.tile([P, NCT], f32, tag="bb", name="bb_sb")
        pp_sb = stp.tile([P, NCT], f32, tag="pp", name="pp_sb")

        def load_w(wT, ct, tag):
            # lhsT tiles: [128 kpart, NKT, 128 m]; block k = wT[kP:(k+1)P, ctP:+P]
            t = wpool.tile([P, NKT, P], bf16, tag=tag, name=tag)
            src = wT[:, :].rearrange("(k p) m -> p k m", p=P)
            nc.sync.dma_start(t[:], src[:, :, ct * P:(ct + 1) * P])
            return t

        for th in range(NTH):
            rws = []
            for ct in range(NCT):
                rw = rwpool.tile([P, NTBH * TB], bf16, tag=f"rw{ct}",
                                 name=f"rw{ct}")
                rws.append(rw)

            for ct in range(NCT):
                wk = load_w(wkT_d, ct, "wk")
                wv = load_w(wvT_d, ct, "wv")
                wr = load_w(wrT_d, ct, "wr")
                lam_b = lam_s[:, ct:ct + 1].broadcast_to([P, TB])
                eu_b = eu_s[:, ct:ct + 1]
                A_prev = B_prev = m_prev = None

                for tbl in range(NTBH):
                    tb = th * NTBH + tbl
                    sl = slice(tb * TB, (tb + 1) * TB)
                    lsl = slice(tbl * TB, (tbl + 1) * TB)

                    kp = pproj.tile([P, TB], f32, tag="kp", name="kp")
                    for k in range(NKT):
                        nc.tensor.matmul(kp[:], wk[:, k, :], xts[k][:, sl],
                                         start=(k == 0), stop=(k == NKT - 1))
                    vp = pproj.tile([P, TB], f32, tag="vp", name="vp")
                    for k in range(NKT):
                        nc.tensor.matmul(vp[:], wv[:, k, :], xts[k][:, sl],
                                         start=(k == 0), stop=(k == NKT - 1))
                    rp = pproj.tile([P, TB], f32, tag="rp", name="rp")
                    for k in range(NKT):
                        nc.tensor.matmul(rp[:], wr[:, k, :], xts[k][:, sl],
                                         start=(k == 0), stop=(k == NKT - 1))

                    if abl_pe_only:
                        continue

                    e_k = work.tile([P, TB], bf16, tag="ek", name="ek")
                    nc.scalar.activation(e_k[:], kp[:], AF.Exp)
                    sg = work.tile([P, TB], bf16, tag="sg", name="sg")
                    nc.scalar.activation(sg[:], rp[:], AF.Sigmoid)
                    pv = work.tile([P, TB], bf16, tag="pv", name="pv")
                    nc.vector.tensor_mul(pv[:], e_k[:], vp[:])

                    # carry-in APs (read directly by scan initial — keeps the
                    # serial chain on the Vector engine, no ACT hop)
                    if tb == 0:
                        initA = a0_s[:, ct:ct + 1]
                        initB = b0_s[:, ct:ct + 1]
                        initM = m0_s[:, ct:ct + 1]
                    elif tbl == 0:
                        initA, initB, initM = (carryA[ct][:], carryB[ct][:],
                                               carryP[ct][:])
                    else:
                        initA = A_prev[:, TB:TB + 1]
                        initB = B_prev[:, TB:TB + 1]
                        initM = m_prev[:, TB - 1:TB]

                    A_blk = scanp.tile([P, TB + 1], f32, tag="A", name="A_blk")
                    B_blk = scanp.tile([P, TB + 1], f32, tag="Bs", name="B_blk")
                    m_blk = scanp.tile([P, TB], f32, tag="ms", name="m_blk")
                    nc.scalar.copy(A_blk[:, 0:1], initA)
                    nc.scalar.copy(B_blk[:, 0:1], initB)

                    if abl_scan_copy:
                        nc.vector.tensor_copy(A_blk[:, 1:TB + 1], pv[:])
                        nc.vector.tensor_copy(B_blk[:, 1:TB + 1], e_k[:])
                        nc.vector.tensor_copy(m_blk[:], e_k[:])
                    else:
                        nc.vector.tensor_tensor_scan(
                            A_blk[:, 1:TB + 1], lam_b, pv[:], initA,
                            OP.mult, OP.add)
                        nc.vector.tensor_tensor_scan(
                            B_blk[:, 1:TB + 1], lam_b, e_k[:], initB,
                            OP.mult, OP.add)
                        # m = e^{pp}: m_t = max(lam*m_{t-1}, e^{k_t})
                        nc.vector.tensor_tensor_scan(
                            m_blk[:], lam_b, e_k[:], initM,
                            OP.mult, OP.max)

                    num = work.tile([P, TB], f32, tag="num", name="num")
                    nc.vector.scalar_tensor_tensor(
                        num[:], pv[:], eu_b, A_blk[:, 0:TB], OP.mult, OP.add)
                    den = work.tile([P, TB], f32, tag="den", name="den")
                    nc.vector.scalar_tensor_tensor(
                        den[:], e_k[:], eu_b, B_blk[:, 0:TB], OP.mult, OP.add)
                    rec = work.tile([P, TB], f32, tag="rec", name="rec")
                    nc.vector.reciprocal_approx_fast(rec[:], den[:])
                    wkv = work.tile([P, TB], bf16, tag="wkv", name="wkv")
                    nc.gpsimd.tensor_mul(wkv[:], num[:], rec[:])
                    nc.gpsimd.tensor_mul(rws[ct][:, lsl], wkv[:], sg[:])

                    A_prev, B_prev, m_prev = A_blk, B_blk, m_blk

                if abl_pe_only:
                    continue
                # save carries at half end
                nc.scalar.copy(carryA[ct][:], A_prev[:, TB:TB + 1])
                nc.scalar.copy(carryB[ct][:], B_prev[:, TB:TB + 1])
                nc.scalar.copy(carryP[ct][:], m_prev[:, TB - 1:TB])

            # partial output projection for this half:
            # outT[d, t] = sum_c woT[c, d] * rwkv[c, t]
            if abl_no_out or abl_pe_only:
                continue
            for dt in range(NDT):
                for tbl in range(NTBH):
                    tb = th * NTBH + tbl
                    op = pout.tile([P, TB], f32, tag="op", name="op")
                    for ctt in range(NCT):
                        nc.tensor.matmul(
                            op[:], wos[ctt][:, dt * P:(dt + 1) * P],
                            rws[ctt][:, tbl * TB:(tbl + 1) * TB],
                            start=(ctt == 0), stop=(ctt == NCT - 1))
                    ob = work.tile([P, TB], bf16, tag="ob", name="ob")
                    nc.scalar.copy(ob[:], op[:])
                    nc.sync.dma_start(
                        outT_d[dt * P:(dt + 1) * P, tb * TB:(tb + 1) * TB],
                        ob[:])

        # final states from carries; carryP holds m_T = e^{pp_T}
        for ct in range(NCT):
            if abl_pe_only:
                break
            rm = stp.tile([P, 1], f32, tag="rm", name="rm")
            nc.vector.reciprocal_approx_fast(rm[:], carryP[ct][:])
            nc.vector.tensor_mul(aa_sb[:, ct:ct + 1], carryA[ct][:], rm[:])
            nc.vector.tensor_mul(bb_sb[:, ct:ct + 1], carryB[ct][:], rm[:])
            nc.scalar.activation(pp_sb[:, ct:ct + 1], carryP[ct][:], AF.Ln)

        nc.sync.dma_start(aa_d[:].rearrange("(j p) -> p j", p=P), aa_sb[:])
        nc.sync.dma_start(bb_d[:].rearrange("(j p) -> p j", p=P), bb_sb[:])
        nc.sync.dma_start(pp_d[:].rearrange("(j p) -> p j", p=P), pp_sb[:])

    nc.finalize()
    return nc


def get_nc():
    global _NC
    if _NC is None:
        _NC = _build_nc()
    return _NC


def make_in_maps(x, key_w, value_w, receptance_w, output_w,
                 time_decay, time_first, aa0, bb0, pp0):
    x = np.asarray(x, F32)
    key_w = np.asarray(key_w, F32)
    value_w = np.asarray(value_w, F32)
    receptance_w = np.asarray(receptance_w, F32)
    output_w = np.asarray(output_w, F32)
    time_decay = np.asarray(time_decay, F32)
    time_first = np.asarray(time_first, F32)
    aa0 = np.asarray(aa0, F32)
    bb0 = np.asarray(bb0, F32)
    pp0 = np.asarray(pp0, F32)

    w = np.exp(time_decay, dtype=F32)
    lam = np.exp(-w, dtype=F32)
    negw = (-w).astype(F32)
    eu = np.exp(time_first, dtype=F32)
    with np.errstate(over="ignore", under="ignore"):
        e0 = np.exp(pp0, dtype=F32)
    A0 = (aa0 * e0).astype(F32)
    B0 = (bb0 * e0).astype(F32)

    xT = [np.ascontiguousarray(x[b].T).astype(BF16) for b in range(B)]
    wT = {}
    for h in range(2):
        cs = slice(h * CSH, (h + 1) * CSH)
        wT[h] = dict(
            wkT=np.ascontiguousarray(key_w[cs, :].T).astype(BF16),
            wvT=np.ascontiguousarray(value_w[cs, :].T).astype(BF16),
            wrT=np.ascontiguousarray(receptance_w[cs, :].T).astype(BF16),
            woT=np.ascontiguousarray(output_w[:, cs].T).astype(BF16),
        )

    in_maps = []
    for core in range(NCORES):
        b, h = divmod(core, 2)
        cs = slice(h * CSH, (h + 1) * CSH)
        in_maps.append(dict(
            xT=xT[b],
            lam=np.ascontiguousarray(lam[cs]),
            negw=np.ascontiguousarray(negw[cs]),
            eu=np.ascontiguousarray(eu[cs]),
            a0=np.ascontiguousarray(A0[b, cs]),
            b0=np.ascontiguousarray(B0[b, cs]),
            p0=np.ascontiguousarray(e0[b, cs]),   # m0 = e^{pp0}
            **wT[h],
        ))
    return in_maps


def unshard(results):
    out = np.empty((B, T, C), F32)
    aa = np.empty((B, C), F32)
    bb = np.empty((B, C), F32)
    pp = np.empty((B, C), F32)
    for b in range(B):
        r0 = results[2 * b]
        r1 = results[2 * b + 1]
        out[b] = (np.asarray(r0["outT"], F32) + np.asarray(r1["outT"], F32)).T
        for h, r in ((0, r0), (1, r1)):
            cs = slice(h * CSH, (h + 1) * CSH)
            aa[b, cs] = np.asarray(r["aaS"], F32)
            bb[b, cs] = np.asarray(r["bbS"], F32)
            pp[b, cs] = np.asarray(r["ppS"], F32)
    return out, aa, bb, pp


def run_spmd(in_maps, trace=False, nc=None, **kwargs):
    from concourse.bass_utils import run_bass_kernel_spmd
    return run_bass_kernel_spmd(nc if nc is not None else get_nc(), in_maps,
                                core_ids=list(range(NCORES)),
                                trace=trace, **kwargs)


def kernel(**inputs):
    in_maps = make_in_maps(**inputs)
    res = run_spmd(in_maps)
    return unshard(res.results)


if __name__ == "__main__":
    nc = get_nc()
    print(f"built: {len(nc.inst_map)} instructions")


# revision 17
# speedup vs baseline: 1.3648x; 1.3648x over previous
"""RWKV time-mixing kernel for 8 Trainium2 NeuronCores (Bass/Tile).

Problem: B=4, T=4096, C=1024.
  k = x @ Wk.T; v = x @ Wv.T; r = sigmoid(x @ Wr.T)
  WKV recurrence over T per (b, channel), out = (r * wkv) @ Wo.T
  plus final recurrence states (aa, bb, pp).

Sharding: 8 cores = (batch b in 0..3) x (channel half h in 0..1).
Each core computes projections for its 512 output channels (contracting
over the full C=1024 input), runs the recurrence with the DVE
tensor_tensor_scan (state = lam*state + e^k*v per channel along T), and
emits a PARTIAL output matmul over its 512 channels. The host sums the
two partials per batch (exact fp32 add) and re-transposes.

Math: the reference's max-stabilized scan is replaced by the equivalent
unstabilized recurrence (safe in fp32 for these magnitudes):
  A_t = lam*A_{t-1} + e^{k_t} v_t,  B_t = lam*B_{t-1} + e^{k_t}
  wkv_t = (A_{t-1} + e^u e^{k_t} v_t) / (B_{t-1} + e^u e^{k_t})
The scan output is written shifted one column so A_{t-1}/B_{t-1} come
for free; num is a fused scalar_tensor_tensor op. den/wkv/rwkv run on
GPSIMD to keep the Vector engine under the PE roofline.
Final states: pp_t = max(pp_{t-1} - w, k_t) is itself a scan (add/max);
aa = A_T * e^{-pp_T}, bb = B_T * e^{-pp_T}.

Engine budget per core (analytic): PE ~230us (4 matmuls, bf16),
DVE ~190us (3 half-rate scans + pv/num/recip), GPSIMD ~125us,
ACT ~95us, DMA ~20MB. PE-bound.
"""

import os
import sys

if "/opt/trn_rl_repo" not in sys.path:
    sys.path.insert(0, "/opt/trn_rl_repo")

import numpy as np
import ml_dtypes

BF16 = ml_dtypes.bfloat16
F32 = np.float32

B, T, C = 4, 4096, 1024
NCORES = 8
CSH = C // 2          # channels per core (512)
P = 128               # partitions
NCT = CSH // P        # 4 channel tiles per core
NKT = C // P          # 8 contraction k-tiles
TB = 512              # T block (one PSUM bank)
NTB = T // TB         # 8
NTH = 1               # out-matmul phases (1 = single phase at the end)
NTBH = NTB // NTH     # blocks per phase
NDT = C // P          # 8 output d-tiles

_NC = None            # cached finalized Bass program


def _build_nc(loop_iters=None):
    import concourse.bacc as bacc
    import concourse.mybir as mybir
    import concourse.tile as tile
    from contextlib import ExitStack

    f32 = mybir.dt.float32
    bf16 = mybir.dt.bfloat16
    AF = mybir.ActivationFunctionType
    OP = mybir.AluOpType

    nc = bacc.Bacc("TRN2")

    xT_d = nc.dram_tensor("xT", [C, T], bf16, kind="ExternalInput")
    wkT_d = nc.dram_tensor("wkT", [C, CSH], bf16, kind="ExternalInput")
    wvT_d = nc.dram_tensor("wvT", [C, CSH], bf16, kind="ExternalInput")
    wrT_d = nc.dram_tensor("wrT", [C, CSH], bf16, kind="ExternalInput")
    woT_d = nc.dram_tensor("woT", [CSH, C], bf16, kind="ExternalInput")
    lam_d = nc.dram_tensor("lam", [CSH], f32, kind="ExternalInput")
    negw_d = nc.dram_tensor("negw", [CSH], f32, kind="ExternalInput")
    eu_d = nc.dram_tensor("eu", [CSH], f32, kind="ExternalInput")
    a0_d = nc.dram_tensor("a0", [CSH], f32, kind="ExternalInput")
    b0_d = nc.dram_tensor("b0", [CSH], f32, kind="ExternalInput")
    p0_d = nc.dram_tensor("p0", [CSH], f32, kind="ExternalInput")

    outT_d = nc.dram_tensor("outT", [C, T], bf16, kind="ExternalOutput")
    aa_d = nc.dram_tensor("aaS", [CSH], f32, kind="ExternalOutput")
    bb_d = nc.dram_tensor("bbS", [CSH], f32, kind="ExternalOutput")
    pp_d = nc.dram_tensor("ppS", [CSH], f32, kind="ExternalOutput")

    with ExitStack() as ctx:
        tc = ctx.enter_context(tile.TileContext(nc))
        cpool = ctx.enter_context(tc.tile_pool(name="const", bufs=1))
        xpool = ctx.enter_context(tc.tile_pool(name="x", bufs=1))
        wpool = ctx.enter_context(tc.tile_pool(name="w", bufs=2))
        wopool = ctx.enter_context(tc.tile_pool(name="wo", bufs=1))
        rwpool = ctx.enter_context(tc.tile_pool(name="rwkv", bufs=1))
        work = ctx.enter_context(tc.tile_pool(name="work", bufs=3))
        scanp = ctx.enter_context(tc.tile_pool(name="scan", bufs=2))
        stp = ctx.enter_context(tc.tile_pool(name="st", bufs=1))
        pproj = ctx.enter_context(tc.tile_pool(name="pproj", bufs=2, space="PSUM"))
        pout = ctx.enter_context(tc.tile_pool(name="pout", bufs=2, space="PSUM"))

        loop_cm = tc.For_i(0, loop_iters, 1) if loop_iters else None
        if loop_cm is not None:
            ctx.enter_context(loop_cm)

        # per-channel constants as [128, NCT] (col = channel tile)
        def load_const(d, tag):
            t = cpool.tile([P, NCT], f32, tag=tag, name=tag)
            nc.sync.dma_start(t[:], d[:].rearrange("(j p) -> p j", p=P))
            return t

        abl_no_out = bool(os.environ.get("ABL_NO_OUT"))
        abl_pe_only = bool(os.environ.get("ABL_PE_ONLY"))
        abl_scan_copy = bool(os.environ.get("ABL_SCAN_COPY"))
        abl_no_gp = bool(os.environ.get("ABL_NO_GP"))

        lam_s = load_const(lam_d, "lam")
        eu_s = load_const(eu_d, "eu")
        a0_s = load_const(a0_d, "a0")
        b0_s = load_const(b0_d, "b0")
        m0_s = load_const(p0_d, "m0")   # host passes m0 = e^{pp0} here

        # resident xT (8 k-tiles of [128, T])
        xts = []
        for k in range(NKT):
            xt = xpool.tile([P, T], bf16, tag=f"xt{k}", name=f"xt{k}")
            nc.sync.dma_start(xt[:], xT_d[k * P:(k + 1) * P, :])
            xts.append(xt)

        # resident output weights per channel tile: [128c, 1024d]
        wos = []
        for ct in range(NCT):
            wo = wopool.tile([P, C], bf16, tag=f"wo{ct}", name=f"wo{ct}")
            nc.sync.dma_start(wo[:], woT_d[ct * P:(ct + 1) * P, :])
            wos.append(wo)

        # carry tiles (recurrence state at half boundaries / end)
        carryA = [stp.tile([P, 1], f32, tag=f"cA{ct}", name=f"cA{ct}")
                  for ct in range(NCT)]
        carryB = [stp.tile([P, 1], f32, tag=f"cB{ct}", name=f"cB{ct}")
                  for ct in range(NCT)]
        carryP = [stp.tile([P, 1], f32, tag=f"cP{ct}", name=f"cP{ct}")
                  for ct in range(NCT)]

        # final state collection tiles
        aa_sb = stp.tile([P, NCT], f32, tag="aa", name="aa_sb")
        bb_sb = stp.tile([P, NCT], f32, tag="bb", name="bb_sb")
        pp_sb = stp.tile([P, NCT], f32, tag="pp", name="pp_sb")

        def load_w(wT, ct, tag):
            # lhsT tiles: [128 kpart, NKT, 128 m]; block k = wT[kP:(k+1)P, ctP:+P]
            t = wpool.tile([P, NKT, P], bf16, tag=tag, name=tag)
            src = wT[:, :].rearrange("(k p) m -> p k m", p=P)
            nc.sync.dma_start(t[:], src[:, :, ct * P:(ct + 1) * P])
            return t

        for th in range(NTH):
            rws = []
            for ct in range(NCT):
                rw = rwpool.tile([P, NTBH * TB], bf16, tag=f"rw{ct}",
                                 name=f"rw{ct}")
                rws.append(rw)

            for ct in range(NCT):
                wk = load_w(wkT_d, ct, "wk")
                wv = load_w(wvT_d, ct, "wv")
                wr = load_w(wrT_d, ct, "wr")
                lam_b = lam_s[:, ct:ct + 1].broadcast_to([P, TB])
                eu_b = eu_s[:, ct:ct + 1]
                A_prev = B_prev = m_prev = None

                for tbl in range(NTBH):
                    tb = th * NTBH + tbl
                    sl = slice(tb * TB, (tb + 1) * TB)
                    lsl = slice(tbl * TB, (tbl + 1) * TB)

                    kp = pproj.tile([P, TB], f32, tag="kp", name="kp")
                    for k in range(NKT):
                        nc.tensor.matmul(kp[:], wk[:, k, :], xts[k][:, sl],
                                         start=(k == 0), stop=(k == NKT - 1))
                    vp = pproj.tile([P, TB], f32, tag="vp", name="vp")
                    for k in range(NKT):
                        nc.tensor.matmul(vp[:], wv[:, k, :], xts[k][:, sl],
                                         start=(k == 0), stop=(k == NKT - 1))
                    rp = pproj.tile([P, TB], f32, tag="rp", name="rp")
                    for k in range(NKT):
                        nc.tensor.matmul(rp[:], wr[:, k, :], xts[k][:, sl],
                                         start=(k == 0), stop=(k == NKT - 1))

                    if abl_pe_only:
                        continue

                    e_k = work.tile([P, TB], bf16, tag="ek", name="ek")
                    nc.scalar.activation(e_k[:], kp[:], AF.Exp)
                    sg = work.tile([P, TB], bf16, tag="sg", name="sg")
                    nc.scalar.activation(sg[:], rp[:], AF.Sigmoid)
                    pv = work.tile([P, TB], bf16, tag="pv", name="pv")
                    nc.vector.tensor_mul(pv[:], e_k[:], vp[:])

                    # carry-in APs (read directly by scan initial — keeps the
                    # serial chain on the Vector engine, no ACT hop)
                    if tb == 0:
                        initA = a0_s[:, ct:ct + 1]
                        initB = b0_s[:, ct:ct + 1]
                        initM = m0_s[:, ct:ct + 1]
                    elif tbl == 0:
                        initA, initB, initM = (carryA[ct][:], carryB[ct][:],
                                               carryP[ct][:])
                    else:
                        initA = A_prev[:, TB:TB + 1]
                        initB = B_prev[:, TB:TB + 1]
                        initM = m_prev[:, TB - 1:TB]

                    A_blk = scanp.tile([P, TB + 1], f32, tag="A", name="A_blk")
                    B_blk = scanp.tile([P, TB + 1], f32, tag="Bs", name="B_blk")
                    m_blk = scanp.tile([P, TB], f32, tag="ms", name="m_blk")
                    nc.scalar.copy(A_blk[:, 0:1], initA)
                    nc.scalar.copy(B_blk[:, 0:1], initB)

                    if abl_scan_copy:
                        nc.vector.tensor_copy(A_blk[:, 1:TB + 1], pv[:])
                        nc.vector.tensor_copy(B_blk[:, 1:TB + 1], e_k[:])
                        nc.vector.tensor_copy(m_blk[:], e_k[:])
                    else:
                        nc.vector.tensor_tensor_scan(
                            A_blk[:, 1:TB + 1], lam_b, pv[:], initA,
                            OP.mult, OP.add)
                        nc.vector.tensor_tensor_scan(
                            B_blk[:, 1:TB + 1], lam_b, e_k[:], initB,
                            OP.mult, OP.add)
                        # m = e^{pp}: m_t = max(lam*m_{t-1}, e^{k_t})
                        nc.vector.tensor_tensor_scan(
                            m_blk[:], lam_b, e_k[:], initM,
                            OP.mult, OP.max)

                    num = work.tile([P, TB], f32, tag="num", name="num")
                    nc.vector.scalar_tensor_tensor(
                        num[:], pv[:], eu_b, A_blk[:, 0:TB], OP.mult, OP.add)
                    den = work.tile([P, TB], f32, tag="den", name="den")
                    nc.vector.scalar_tensor_tensor(
                        den[:], e_k[:], eu_b, B_blk[:, 0:TB], OP.mult, OP.add)
                    rec = work.tile([P, TB], f32, tag="rec", name="rec")
                    nc.vector.reciprocal_approx_fast(rec[:], den[:])
                    if not abl_no_gp:
                        wkv = work.tile([P, TB], bf16, tag="wkv", name="wkv")
                        nc.gpsimd.tensor_mul(wkv[:], num[:], rec[:])
                        nc.gpsimd.tensor_mul(rws[ct][:, lsl], wkv[:], sg[:])

                    A_prev, B_prev, m_prev = A_blk, B_blk, m_blk

                if abl_pe_only:
                    continue
                # save carries at half end
                nc.scalar.copy(carryA[ct][:], A_prev[:, TB:TB + 1])
                nc.scalar.copy(carryB[ct][:], B_prev[:, TB:TB + 1])
                nc.scalar.copy(carryP[ct][:], m_prev[:, TB - 1:TB])

            # partial output projection for this half:
            # outT[d, t] = sum_c woT[c, d] * rwkv[c, t]
            if abl_no_out or abl_pe_only or abl_no_gp:
                continue
            for dt in range(NDT):
                for tbl in range(NTBH):
                    tb = th * NTBH + tbl
                    op = pout.tile([P, TB], f32, tag="op", name="op")
                    for ctt in range(NCT):
                        nc.tensor.matmul(
                            op[:], wos[ctt][:, dt * P:(dt + 1) * P],
                            rws[ctt][:, tbl * TB:(tbl + 1) * TB],
                            start=(ctt == 0), stop=(ctt == NCT - 1))
                    ob = work.tile([P, TB], bf16, tag="ob", name="ob")
                    nc.scalar.copy(ob[:], op[:])
                    nc.sync.dma_start(
                        outT_d[dt * P:(dt + 1) * P, tb * TB:(tb + 1) * TB],
                        ob[:])

        # final states from carries; carryP holds m_T = e^{pp_T}
        if abl_pe_only:
            nc.vector.memset(aa_sb[:], 0.0)
            nc.vector.memset(bb_sb[:], 0.0)
            nc.vector.memset(pp_sb[:], 0.0)
        for ct in range(NCT):
            if abl_pe_only:
                break
            rm = stp.tile([P, 1], f32, tag="rm", name="rm")
            nc.vector.reciprocal_approx_fast(rm[:], carryP[ct][:])
            nc.vector.tensor_mul(aa_sb[:, ct:ct + 1], carryA[ct][:], rm[:])
            nc.vector.tensor_mul(bb_sb[:, ct:ct + 1], carryB[ct][:], rm[:])
            nc.scalar.activation(pp_sb[:, ct:ct + 1], carryP[ct][:], AF.Ln)

        nc.sync.dma_start(aa_d[:].rearrange("(j p) -> p j", p=P), aa_sb[:])
        nc.sync.dma_start(bb_d[:].rearrange("(j p) -> p j", p=P), bb_sb[:])
        nc.sync.dma_start(pp_d[:].rearrange("(j p) -> p j", p=P), pp_sb[:])

    nc.finalize()
    return nc


def get_nc():
    global _NC
    if _NC is None:
        _NC = _build_nc()
    return _NC


def make_in_maps(x, key_w, value_w, receptance_w, output_w,
                 time_decay, time_first, aa0, bb0, pp0):
    x = np.asarray(x, F32)
    key_w = np.asarray(key_w, F32)
    value_w = np.asarray(value_w, F32)
    receptance_w = np.asarray(receptance_w, F32)
    output_w = np.asarray(output_w, F32)
    time_decay = np.asarray(time_decay, F32)
    time_first = np.asarray(time_first, F32)
    aa0 = np.asarray(aa0, F32)
    bb0 = np.asarray(bb0, F32)
    pp0 = np.asarray(pp0, F32)

    w = np.exp(time_decay, dtype=F32)
    lam = np.exp(-w, dtype=F32)
    negw = (-w).astype(F32)
    eu = np.exp(time_first, dtype=F32)
    with np.errstate(over="ignore", under="ignore"):
        e0 = np.exp(pp0, dtype=F32)
    A0 = (aa0 * e0).astype(F32)
    B0 = (bb0 * e0).astype(F32)

    xT = [np.ascontiguousarray(x[b].T).astype(BF16) for b in range(B)]
    wT = {}
    for h in range(2):
        cs = slice(h * CSH, (h + 1) * CSH)
        wT[h] = dict(
            wkT=np.ascontiguousarray(key_w[cs, :].T).astype(BF16),
            wvT=np.ascontiguousarray(value_w[cs, :].T).astype(BF16),
            wrT=np.ascontiguousarray(receptance_w[cs, :].T).astype(BF16),
            woT=np.ascontiguousarray(output_w[:, cs].T).astype(BF16),
        )

    in_maps = []
    for core in range(NCORES):
        b, h = divmod(core, 2)
        cs = slice(h * CSH, (h + 1) * CSH)
        in_maps.append(dict(
            xT=xT[b],
            lam=np.ascontiguousarray(lam[cs]),
            negw=np.ascontiguousarray(negw[cs]),
            eu=np.ascontiguousarray(eu[cs]),
            a0=np.ascontiguousarray(A0[b, cs]),
            b0=np.ascontiguousarray(B0[b, cs]),
            p0=np.ascontiguousarray(e0[b, cs]),   # m0 = e^{pp0}
            **wT[h],
        ))
    return in_maps


def unshard(results):
    out = np.empty((B, T, C), F32)
    aa = np.empty((B, C), F32)
    bb = np.empty((B, C), F32)
    pp = np.empty((B, C), F32)
    for b in range(B):
        r0 = results[2 * b]
        r1 = results[2 * b + 1]
        out[b] = (np.asarray(r0["outT"], F32) + np.asarray(r1["outT"], F32)).T
        for h, r in ((0, r0), (1, r1)):
            cs = slice(h * CSH, (h + 1) * CSH)
            aa[b, cs] = np.asarray(r["aaS"], F32)
            bb[b, cs] = np.asarray(r["bbS"], F32)
            pp[b, cs] = np.asarray(r["ppS"], F32)
    return out, aa, bb, pp


def run_spmd(in_maps, trace=False, nc=None, **kwargs):
    from concourse.bass_utils import run_bass_kernel_spmd
    return run_bass_kernel_spmd(nc if nc is not None else get_nc(), in_maps,
                                core_ids=list(range(NCORES)),
                                trace=trace, **kwargs)


def kernel(**inputs):
    in_maps = make_in_maps(**inputs)
    res = run_spmd(in_maps)
    return unshard(res.results)


if __name__ == "__main__":
    nc = get_nc()
    print(f"built: {len(nc.inst_map)} instructions")
